# revision 30
# baseline (speedup 1.0000x reference)
"""Trainium2 Bass kernel for AMM (landmark/Nystrom-style) attention.

Per batch element (8 total, one NeuronCore each):
    qkv  = x @ W_qkv; q,k,v = split(qkv); q /= sqrt(512)
    keys_lm = segment_mean(k, 16); vals_lm = segment_mean(v, 16)
    out  = softmax(q @ keys_lm^T) @ vals_lm @ W_proj + b_proj
    return v + out

Algebraic restructuring (exact in real arithmetic):
  - segment_mean commutes with the projections: keys_lm = pool(x) @ W_k,
    vals_lm = pool(x) @ W_v  -> the full k matmul is never computed.
  - THE QUERY MATMUL IS NEVER COMPUTED EITHER:
        scores = (x @ W_q) @ keys_lm^T = x @ (W_q @ keys_lm^T)
    and W_q @ keys_lm^T is only 512x256 - computed once per core. This
    removes the 4096x512x512 q projection entirely.
  - attn @ vals_lm @ W_proj -> attn @ (vals_lm @ W_proj): 256 rows through
    W_proj instead of 4096.
  - b_proj folded into VW by a rank-1 matmul (softmax rows sum to 1).
  - softmax normalization applied after the value matmul:
        out = (E @ VWb) / (E @ 1),  E = exp(logits).

Per-core TensorE work: v (128 MMs) + scores (64) + out2 (64) + den (64 N=1)
+ landmark-side (~60 small) ~= 66us of bf16 matmul at 1 cycle/row.

Sharding: pure data-parallel over batch B=8 across 8 cores, weights
replicated, no collectives. Host pre-transposes x per core (channel dim on
partitions) and casts matmul inputs to bf16 (fp32 PSUM accumulation, fp32
output). rel err vs the fp32 reference ~2.4e-3 (gate 2e-2).

Engine layout (HW-tuned): program order puts the v matmuls first so the PE
is busy while x streams in; the landmark chain (pool -> keys/vals -> KW/VW)
overlaps it; the attention phase is scores -> exp(ACT) -> den/out2 ->
fused (out2*r + v) on DVE -> output DMA. All ACT Copy ops precede all Exp
ops (one activation-table switch). fp8/DoubleRow measured slower in-context
on this HW; everything stays bf16.
"""

import sys
from contextlib import ExitStack

import numpy as np

sys.path.insert(0, "/opt/trn_rl_repo")

import concourse.bass as bass  # noqa: E402
import concourse.tile as tile  # noqa: E402
from concourse import bacc, mybir  # noqa: E402
from concourse.bass_utils import run_bass_kernel_spmd  # noqa: E402

import ml_dtypes  # noqa: E402

BF16 = mybir.dt.bfloat16
F32 = mybir.dt.float32
AF = mybir.ActivationFunctionType
ALU = mybir.AluOpType

B, N, DIM = 8, 4096, 512
L, SEG = 256, 16
CT = DIM // 128
MT = N // 512

XT_CHUNK = 1024  # width of x DMA chunks (2KB rows = one DMA descriptor/run)
RSCALE = float(1.0 / np.sqrt(512.0))


def build_kernel(ctx: ExitStack, tc: "tile.TileContext", out_d, xt_d, wkv_d, wqT_d, wproj_d, bproj_d):
    nc = tc.nc

    consts = ctx.enter_context(tc.tile_pool(name="consts", bufs=1))
    work = ctx.enter_context(tc.tile_pool(name="work", bufs=3))
    psum = ctx.enter_context(tc.tile_pool(name="psum", bufs=4, space="PSUM"))
    psumv = ctx.enter_context(tc.tile_pool(name="psumv", bufs=2, space="PSUM"))
    psden = ctx.enter_context(tc.tile_pool(name="psden", bufs=2, space="PSUM"))

    # ---- weights ------------------------------------------------------------
    wkv = consts.tile([128, CT, 2 * DIM], BF16)  # [c_lo, cj, (k|v) columns]
    for j in range(CT):
        nc.sync.dma_start(out=wkv[:, j, :], in_=wkv_d[j, :, :])

    ones_col = consts.tile([128, 1], BF16)
    nc.vector.memset(ones_col[:, :], 1.0)
    ones_row = consts.tile([1, 128], BF16)
    nc.vector.memset(ones_row[:, :], 1.0)

    # ---- x^T (chunked DMA) + segment pooling on DVE -------------------------
    xt = consts.tile([128, CT, N], BF16)
    poolf = consts.tile([128, CT, L], F32)  # segment sums (= 16*mean), f32
    xpool = consts.tile([128, CT, L], BF16)
    NCH = N // XT_CHUNK
    LC = L // NCH
    for ci in range(NCH):
        c0, c1 = ci * XT_CHUNK, (ci + 1) * XT_CHUNK
        for j in range(CT):
            nc.sync.dma_start(out=xt[:, j, c0:c1], in_=xt_d[j, :, c0:c1])
        for j in range(CT):
            nc.vector.reduce_sum(
                poolf[:, j, ci * LC : (ci + 1) * LC],
                xt[:, j, c0:c1].rearrange("p (l s) -> p l s", s=SEG),
                axis=mybir.AxisListType.X,
            )
    for j in range(CT):
        nc.vector.tensor_scalar_mul(xpool[:, j, :], poolf[:, j, :], 1.0 / SEG)

    wqT = consts.tile([128, CT, DIM], BF16)  # W_q^T  [d_lo, dj, c]
    for j in range(CT):
        nc.sync.dma_start(out=wqT[:, j, :], in_=wqT_d[j, :, :])
    wproj = consts.tile([128, CT, DIM], BF16)
    for j in range(CT):
        nc.sync.dma_start(out=wproj[:, j, :], in_=wproj_d[j, :, :])
    bproj = consts.tile([1, DIM], BF16)
    nc.sync.dma_start(out=bproj[:, :], in_=bproj_d[:, :])

    # ---- v = x @ W_v, natural layout, first in program order so the PE is
    # busy from the first x chunk while the landmark chain waits on pooling.
    vstore = consts.tile([128, 32, 512], BF16)
    for mi in range(MT):
        for t in range(4):
            r0 = mi * 512 + t * 128
            vp = psumv.tile([128, 512], F32, tag="mmv")
            for cj in range(CT):
                nc.tensor.matmul(
                    vp[:, :],
                    xt[:, cj, r0 : r0 + 128],
                    wkv[:, cj, DIM : 2 * DIM],
                    start=(cj == 0),
                    stop=(cj == CT - 1),
                )
            # all v copies on DVE: ACT runs in program order, and parking v
            # copies there would block the landmark-chain copies (and hence
            # the first scores matmul) behind the whole v phase.
            nc.vector.tensor_copy(vstore[:, mi * 4 + t, :], vp[:, :])

    # ---- landmark projections: keys_lm^T, vals_lm^T [d_lo, dj, l] ----------
    keysT = consts.tile([128, CT, L], BF16)
    valsT = consts.tile([128, CT, L], BF16)
    for dst, col0 in ((keysT, 0), (valsT, DIM)):
        for dj in range(CT):
            pt = psum.tile([128, L], F32, tag="mm")
            for cj in range(CT):
                nc.tensor.matmul(
                    pt[:, :],
                    wkv[:, cj, col0 + dj * 128 : col0 + (dj + 1) * 128],
                    xpool[:, cj, :],
                    start=(cj == 0),
                    stop=(cj == CT - 1),
                )
            nc.scalar.copy(dst[:, dj, :], pt[:, :])

    # ---- KW = W_q @ keys_lm^T  [c_lo, cj, l]  (the q-projection absorber) --
    kw = consts.tile([128, CT, L], BF16)
    for cj in range(CT):
        pt = psum.tile([128, L], F32, tag="mm")
        for dj in range(CT):
            nc.tensor.matmul(
                pt[:, :],
                wqT[:, dj, cj * 128 : (cj + 1) * 128],
                keysT[:, dj, :],
                start=(dj == 0),
                stop=(dj == CT - 1),
            )
        nc.scalar.copy(kw[:, cj, :], pt[:, :])

    # ---- VWb = vals_lm @ W_proj + 1b  [l_lo, li, d] -------------------------
    vw = consts.tile([128, 2, DIM], BF16)
    for li in range(2):
        pt = psum.tile([128, DIM], F32, tag="mm")
        for dj in range(CT):
            nc.tensor.matmul(
                pt[:, :],
                valsT[:, dj, li * 128 : (li + 1) * 128],
                wproj[:, dj, :],
                start=(dj == 0),
                stop=False,
            )
        nc.tensor.matmul(pt[:, :], ones_row[:, :], bproj[:, :], start=False, stop=True)
        nc.scalar.copy(vw[:, li, :], pt[:, :])

    # ---- attention: scoresT = KW^T-contracted directly against x^T ----------
    # software-pipelined: scores(mi+1) is issued before out2/den(mi) so the
    # PE never stalls on the exp (ACT) latency.
    def do_scores(mi):
        et = work.tile([128, 2, 512], BF16, tag="et")  # E = exp(logits/sqrt(512))
        for li in range(2):
            pt = psum.tile([128, 512], F32, tag="mm")
            for cj in range(CT):
                nc.tensor.matmul(
                    pt[:, :],
                    kw[:, cj, li * 128 : (li + 1) * 128],
                    xt[:, cj, mi * 512 : (mi + 1) * 512],
                    start=(cj == 0),
                    stop=(cj == CT - 1),
                )
            nc.scalar.activation(et[:, li, :], pt[:, :], AF.Exp, scale=RSCALE)
        return et

    et_next = do_scores(0)
    for mi in range(MT):
        et = et_next
        if mi + 1 < MT:
            et_next = do_scores(mi + 1)

        for t in range(4):
            r0 = mi * 512 + t * 128
            sl = slice(t * 128, (t + 1) * 128)
            # out2 = E @ VWb, then den with the same stationary weights
            op = psum.tile([128, 512], F32, tag="mm")
            dp = psden.tile([128, 1], F32, tag="den")
            for li in range(2):
                nc.tensor.matmul(
                    op[:, :], et[:, li, sl], vw[:, li, :],
                    start=(li == 0), stop=(li == 1),
                )
            for li in range(2):
                nc.tensor.matmul(
                    dp[:, :], et[:, li, sl], ones_col[:, :],
                    start=(li == 0), stop=(li == 1),
                )
            rr = work.tile([128, 1], F32, tag="rr")
            nc.vector.reciprocal(rr[:, :], dp[:, :])
            fin = work.tile([128, 512], F32, tag="fin")
            nc.vector.scalar_tensor_tensor(
                fin[:, :], op[:, :], rr[:, :], vstore[:, mi * 4 + t, :],
                op0=ALU.mult, op1=ALU.add,
            )
            nc.sync.dma_start(out=out_d[r0 : r0 + 128, :], in_=fin[:, :])


def build_nc(repeat: int = 1):
    nc = bacc.Bacc("TRN2", target_bir_lowering=False, debug=False, num_devices=8)
    xt_d = nc.declare_dram_parameter("xt", [CT, 128, N], BF16, isOutput=False)
    wkv_d = nc.declare_dram_parameter("wkv", [CT, 128, 2 * DIM], BF16, isOutput=False)
    wqT_d = nc.declare_dram_parameter("wqT", [CT, 128, DIM], BF16, isOutput=False)
    wproj_d = nc.declare_dram_parameter("wproj", [CT, 128, DIM], BF16, isOutput=False)
    bproj_d = nc.declare_dram_parameter("bproj", [1, DIM], BF16, isOutput=False)
    out_d = nc.declare_dram_parameter("out", [N, DIM], F32, isOutput=True)
    aps = (out_d.ap(), xt_d.ap(), wkv_d.ap(), wqT_d.ap(), wproj_d.ap(), bproj_d.ap())
    with tile.TileContext(nc) as tc, ExitStack() as ctx:
        if repeat == 1:
            build_kernel(ctx, tc, *aps)
        else:
            with tc.For_i(0, repeat, 1):
                build_kernel(ctx, tc, *aps)
    nc.compile()
    return nc


def prep_in_maps(x, W_qkv, W_proj, b_proj):
    bf = ml_dtypes.bfloat16
    W_qkv = np.asarray(W_qkv, np.float32)
    wkv = np.ascontiguousarray(W_qkv[:, DIM:].astype(bf).reshape(CT, 128, 2 * DIM))
    wqT = np.ascontiguousarray(W_qkv[:, :DIM].T.astype(bf)).reshape(CT, 128, DIM)
    wp = np.ascontiguousarray(np.asarray(W_proj, np.float32).astype(bf).reshape(CT, 128, DIM))
    bp = np.asarray(b_proj, np.float32).astype(bf).reshape(1, DIM)
    in_maps = []
    for i in range(B):
        xti = np.ascontiguousarray(np.asarray(x[i], np.float32).T.astype(bf)).reshape(CT, 128, N)
        in_maps.append({"xt": xti, "wkv": wkv, "wqT": wqT, "wproj": wp, "bproj": bp})
    return in_maps


_NC_CACHE = None


def kernel(x, W_qkv, W_proj, b_proj):
    global _NC_CACHE
    if _NC_CACHE is None:
        _NC_CACHE = build_nc()
    nc = _NC_CACHE
    in_maps = prep_in_maps(x, W_qkv, W_proj, b_proj)
    res = run_bass_kernel_spmd(nc, in_maps, core_ids=list(range(B)))
    out = np.stack([res.results[i]["out"] for i in range(B)], axis=0)
    return out.astype(np.float32)


# revision 34
# speedup vs baseline: 1.2169x; 1.2169x over previous
"""Trainium2 Bass kernel for AMM (landmark/Nystrom-style) attention.

Per batch element (8 total, one NeuronCore each):
    qkv  = x @ W_qkv; q,k,v = split(qkv); q /= sqrt(512)
    keys_lm = segment_mean(k, 16); vals_lm = segment_mean(v, 16)
    out  = softmax(q @ keys_lm^T) @ vals_lm @ W_proj + b_proj
    return v + out

Algebraic restructuring (exact in real arithmetic):
  - segment_mean commutes with the projections: keys_lm = pool(x) @ W_k,
    vals_lm = pool(x) @ W_v  -> the full k matmul is never computed.
  - THE QUERY MATMUL IS NEVER COMPUTED EITHER:
        scores = (x @ W_q) @ keys_lm^T = x @ (W_q @ keys_lm^T)
    and W_q @ keys_lm^T is only 512x256 - computed once per core. This
    removes the 4096x512x512 q projection entirely.
  - attn @ vals_lm @ W_proj -> attn @ (vals_lm @ W_proj): 256 rows through
    W_proj instead of 4096.
  - b_proj folded into VW by a rank-1 matmul (softmax rows sum to 1).
  - softmax normalization applied after the value matmul:
        out = (E @ VWb) / (E @ 1),  E = exp(logits).

Per-core TensorE work: v (128 MMs) + scores (64) + out2 (64) + den (64 N=1)
+ landmark-side (~60 small) ~= 66us of bf16 matmul at 1 cycle/row.

Sharding: pure data-parallel over batch B=8 across 8 cores, weights
replicated, no collectives. Host pre-transposes x per core (channel dim on
partitions) and casts matmul inputs to bf16 (fp32 PSUM accumulation, fp32
output). rel err vs the fp32 reference ~2.4e-3 (gate 2e-2).

Engine layout (HW-tuned): program order puts the v matmuls first so the PE
is busy while x streams in; the landmark chain (pool -> keys/vals -> KW/VW)
overlaps it; the attention phase is scores -> exp(ACT) -> den/out2 ->
fused (out2*r + v) on DVE -> output DMA. All ACT Copy ops precede all Exp
ops (one activation-table switch). fp8/DoubleRow measured slower in-context
on this HW; everything stays bf16.
"""

import sys
from contextlib import ExitStack

import numpy as np

sys.path.insert(0, "/opt/trn_rl_repo")

import concourse.bass as bass  # noqa: E402
import concourse.tile as tile  # noqa: E402
from concourse import bacc, mybir  # noqa: E402
from concourse.bass_utils import run_bass_kernel_spmd  # noqa: E402

import ml_dtypes  # noqa: E402

BF16 = mybir.dt.bfloat16
F32 = mybir.dt.float32
AF = mybir.ActivationFunctionType
ALU = mybir.AluOpType

B, N, DIM = 8, 4096, 512
L, SEG = 256, 16
CT = DIM // 128
MT = N // 512

XT_CHUNK = 1024  # width of x DMA chunks (2KB rows = one DMA descriptor/run)
RSCALE = float(1.0 / np.sqrt(512.0))


def build_kernel(ctx: ExitStack, tc: "tile.TileContext", out_d, xt_d, wkv_d, wqT_d, wproj_d, bproj_d):
    nc = tc.nc

    consts = ctx.enter_context(tc.tile_pool(name="consts", bufs=1))
    work = ctx.enter_context(tc.tile_pool(name="work", bufs=3))
    psum = ctx.enter_context(tc.tile_pool(name="psum", bufs=4, space="PSUM"))
    psumv = ctx.enter_context(tc.tile_pool(name="psumv", bufs=2, space="PSUM"))
    psden = ctx.enter_context(tc.tile_pool(name="psden", bufs=2, space="PSUM"))

    # ---- weights ------------------------------------------------------------
    wkv = consts.tile([128, CT, 2 * DIM], BF16)  # [c_lo, cj, (k|v) columns]
    for j in range(CT):
        nc.sync.dma_start(out=wkv[:, j, :], in_=wkv_d[j, :, :])

    ones_col = consts.tile([128, 1], BF16)
    nc.vector.memset(ones_col[:, :], 1.0)
    ones_row = consts.tile([1, 128], BF16)
    nc.vector.memset(ones_row[:, :], 1.0)

    # ---- x^T (chunked DMA) + segment pooling on DVE -------------------------
    xt = consts.tile([128, CT, N], BF16)
    poolf = consts.tile([128, CT, L], F32)  # segment sums (= 16*mean), f32
    xpool = consts.tile([128, CT, L], BF16)
    NCH = N // XT_CHUNK
    LC = L // NCH
    for ci in range(NCH):
        c0, c1 = ci * XT_CHUNK, (ci + 1) * XT_CHUNK
        for j in range(CT):
            nc.sync.dma_start(out=xt[:, j, c0:c1], in_=xt_d[j, :, c0:c1])
        for j in range(CT):
            nc.vector.reduce_sum(
                poolf[:, j, ci * LC : (ci + 1) * LC],
                xt[:, j, c0:c1].rearrange("p (l s) -> p l s", s=SEG),
                axis=mybir.AxisListType.X,
            )
    for j in range(CT):
        nc.vector.tensor_scalar_mul(xpool[:, j, :], poolf[:, j, :], 1.0 / SEG)

    wqT = consts.tile([128, CT, DIM], BF16)  # W_q^T  [d_lo, dj, c]
    for j in range(CT):
        nc.sync.dma_start(out=wqT[:, j, :], in_=wqT_d[j, :, :])
    wproj = consts.tile([128, CT, DIM], BF16)
    for j in range(CT):
        nc.sync.dma_start(out=wproj[:, j, :], in_=wproj_d[j, :, :])
    bproj = consts.tile([1, DIM], BF16)
    nc.sync.dma_start(out=bproj[:, :], in_=bproj_d[:, :])

    # ---- v = x @ W_v, natural layout, first in program order so the PE is
    # busy from the first x chunk while the landmark chain waits on pooling.
    vstore = consts.tile([128, 32, 512], BF16)
    for mi in range(MT):
        for t in range(4):
            r0 = mi * 512 + t * 128
            vp = psumv.tile([128, 512], F32, tag="mmv")
            for cj in range(CT):
                nc.tensor.matmul(
                    vp[:, :],
                    xt[:, cj, r0 : r0 + 128],
                    wkv[:, cj, DIM : 2 * DIM],
                    start=(cj == 0),
                    stop=(cj == CT - 1),
                )
            # v copies on ACT (idle until the landmark chain at ~24us): on
            # DVE they queue behind all 16 pooling reduces, starving the
            # 2-slot v PSUM pool and stalling the PE 14.8us (trace-measured).
            # They precede the landmark copies in ACT order and finish
            # chasing the v matmuls, so the landmark chain is not delayed.
            nc.scalar.copy(vstore[:, mi * 4 + t, :], vp[:, :])

    # ---- landmark projections: keys_lm^T, vals_lm^T [d_lo, dj, l] ----------
    keysT = consts.tile([128, CT, L], BF16)
    valsT = consts.tile([128, CT, L], BF16)
    for dst, col0 in ((keysT, 0), (valsT, DIM)):
        for dj in range(CT):
            pt = psum.tile([128, L], F32, tag="mm")
            for cj in range(CT):
                nc.tensor.matmul(
                    pt[:, :],
                    wkv[:, cj, col0 + dj * 128 : col0 + (dj + 1) * 128],
                    xpool[:, cj, :],
                    start=(cj == 0),
                    stop=(cj == CT - 1),
                )
            nc.scalar.copy(dst[:, dj, :], pt[:, :])

    # ---- KW = W_q @ keys_lm^T  [c_lo, cj, l]  (the q-projection absorber) --
    kw = consts.tile([128, CT, L], BF16)
    for cj in range(CT):
        pt = psum.tile([128, L], F32, tag="mm")
        for dj in range(CT):
            nc.tensor.matmul(
                pt[:, :],
                wqT[:, dj, cj * 128 : (cj + 1) * 128],
                keysT[:, dj, :],
                start=(dj == 0),
                stop=(dj == CT - 1),
            )
        nc.scalar.copy(kw[:, cj, :], pt[:, :])

    # ---- VWb = vals_lm @ W_proj + 1b  [l_lo, li, d] -------------------------
    vw = consts.tile([128, 2, DIM], BF16)
    for li in range(2):
        pt = psum.tile([128, DIM], F32, tag="mm")
        for dj in range(CT):
            nc.tensor.matmul(
                pt[:, :],
                valsT[:, dj, li * 128 : (li + 1) * 128],
                wproj[:, dj, :],
                start=(dj == 0),
                stop=False,
            )
        nc.tensor.matmul(pt[:, :], ones_row[:, :], bproj[:, :], start=False, stop=True)
        nc.scalar.copy(vw[:, li, :], pt[:, :])

    # ---- attention: scoresT = KW^T-contracted directly against x^T ----------
    # software-pipelined: scores(mi+1) is issued before out2/den(mi) so the
    # PE never stalls on the exp (ACT) latency.
    def do_scores(mi):
        et = work.tile([128, 2, 512], BF16, tag="et")  # E = exp(logits/sqrt(512))
        for li in range(2):
            pt = psum.tile([128, 512], F32, tag="mm")
            for cj in range(CT):
                nc.tensor.matmul(
                    pt[:, :],
                    kw[:, cj, li * 128 : (li + 1) * 128],
                    xt[:, cj, mi * 512 : (mi + 1) * 512],
                    start=(cj == 0),
                    stop=(cj == CT - 1),
                )
            nc.scalar.activation(et[:, li, :], pt[:, :], AF.Exp, scale=RSCALE)
        return et

    et_next = do_scores(0)
    for mi in range(MT):
        et = et_next
        if mi + 1 < MT:
            et_next = do_scores(mi + 1)

        for t in range(4):
            r0 = mi * 512 + t * 128
            sl = slice(t * 128, (t + 1) * 128)
            # out2 = E @ VWb, then den with the same stationary weights
            op = psum.tile([128, 512], F32, tag="mm")
            dp = psden.tile([128, 1], F32, tag="den")
            for li in range(2):
                nc.tensor.matmul(
                    op[:, :], et[:, li, sl], vw[:, li, :],
                    start=(li == 0), stop=(li == 1),
                )
            for li in range(2):
                nc.tensor.matmul(
                    dp[:, :], et[:, li, sl], ones_col[:, :],
                    start=(li == 0), stop=(li == 1),
                )
            rr = work.tile([128, 1], F32, tag="rr")
            nc.vector.reciprocal(rr[:, :], dp[:, :])
            fin = work.tile([128, 512], F32, tag="fin")
            nc.vector.scalar_tensor_tensor(
                fin[:, :], op[:, :], rr[:, :], vstore[:, mi * 4 + t, :],
                op0=ALU.mult, op1=ALU.add,
            )
            nc.sync.dma_start(out=out_d[r0 : r0 + 128, :], in_=fin[:, :])


def build_nc(repeat: int = 1):
    nc = bacc.Bacc("TRN2", target_bir_lowering=False, debug=False, num_devices=8)
    xt_d = nc.declare_dram_parameter("xt", [CT, 128, N], BF16, isOutput=False)
    wkv_d = nc.declare_dram_parameter("wkv", [CT, 128, 2 * DIM], BF16, isOutput=False)
    wqT_d = nc.declare_dram_parameter("wqT", [CT, 128, DIM], BF16, isOutput=False)
    wproj_d = nc.declare_dram_parameter("wproj", [CT, 128, DIM], BF16, isOutput=False)
    bproj_d = nc.declare_dram_parameter("bproj", [1, DIM], BF16, isOutput=False)
    out_d = nc.declare_dram_parameter("out", [N, DIM], F32, isOutput=True)
    aps = (out_d.ap(), xt_d.ap(), wkv_d.ap(), wqT_d.ap(), wproj_d.ap(), bproj_d.ap())
    with tile.TileContext(nc) as tc, ExitStack() as ctx:
        if repeat == 1:
            build_kernel(ctx, tc, *aps)
        else:
            with tc.For_i(0, repeat, 1):
                build_kernel(ctx, tc, *aps)
    nc.compile()
    return nc


def prep_in_maps(x, W_qkv, W_proj, b_proj):
    bf = ml_dtypes.bfloat16
    W_qkv = np.asarray(W_qkv, np.float32)
    wkv = np.ascontiguousarray(W_qkv[:, DIM:].astype(bf).reshape(CT, 128, 2 * DIM))
    wqT = np.ascontiguousarray(W_qkv[:, :DIM].T.astype(bf)).reshape(CT, 128, DIM)
    wp = np.ascontiguousarray(np.asarray(W_proj, np.float32).astype(bf).reshape(CT, 128, DIM))
    bp = np.asarray(b_proj, np.float32).astype(bf).reshape(1, DIM)
    in_maps = []
    for i in range(B):
        xti = np.ascontiguousarray(np.asarray(x[i], np.float32).T.astype(bf)).reshape(CT, 128, N)
        in_maps.append({"xt": xti, "wkv": wkv, "wqT": wqT, "wproj": wp, "bproj": bp})
    return in_maps


_NC_CACHE = None


def kernel(x, W_qkv, W_proj, b_proj):
    global _NC_CACHE
    if _NC_CACHE is None:
        _NC_CACHE = build_nc()
    nc = _NC_CACHE
    in_maps = prep_in_maps(x, W_qkv, W_proj, b_proj)
    res = run_bass_kernel_spmd(nc, in_maps, core_ids=list(range(B)))
    out = np.stack([res.results[i]["out"] for i in range(B)], axis=0)
    return out.astype(np.float32)


# revision 35
# speedup vs baseline: 30.4914x; 25.0571x over previous
"""Trainium2 Bass kernel for AMM (landmark/Nystrom-style) attention.

Per batch element (8 total, one NeuronCore each):
    qkv  = x @ W_qkv; q,k,v = split(qkv); q /= sqrt(512)
    keys_lm = segment_mean(k, 16); vals_lm = segment_mean(v, 16)
    out  = softmax(q @ keys_lm^T) @ vals_lm @ W_proj + b_proj
    return v + out

Algebraic restructuring (exact in real arithmetic):
  - segment_mean commutes with the projections: keys_lm = pool(x) @ W_k,
    vals_lm = pool(x) @ W_v  -> the full k matmul is never computed.
  - THE QUERY MATMUL IS NEVER COMPUTED EITHER:
        scores = (x @ W_q) @ keys_lm^T = x @ (W_q @ keys_lm^T)
    and W_q @ keys_lm^T is only 512x256 - computed once per core. This
    removes the 4096x512x512 q projection entirely.
  - attn @ vals_lm @ W_proj -> attn @ (vals_lm @ W_proj): 256 rows through
    W_proj instead of 4096.
  - b_proj folded into VW by a rank-1 matmul (softmax rows sum to 1).
  - softmax normalization applied after the value matmul:
        out = (E @ VWb) / (E @ 1),  E = exp(logits).

Per-core TensorE work: v (128 MMs) + scores (64) + out2 (64) + den (64 N=1)
+ landmark-side (~60 small) ~= 66us of bf16 matmul at 1 cycle/row.

Sharding: pure data-parallel over batch B=8 across 8 cores, weights
replicated, no collectives. Host pre-transposes x per core (channel dim on
partitions) and casts matmul inputs to bf16 (fp32 PSUM accumulation, fp32
output). rel err vs the fp32 reference ~2.4e-3 (gate 2e-2).

Engine layout (HW-tuned): program order puts the v matmuls first so the PE
is busy while x streams in; the landmark chain (pool -> keys/vals -> KW/VW)
overlaps it; the attention phase is scores -> exp(ACT) -> den/out2 ->
fused (out2*r + v) on DVE -> output DMA. All ACT Copy ops precede all Exp
ops (one activation-table switch). fp8/DoubleRow measured slower in-context
on this HW; everything stays bf16.
"""

import sys
from contextlib import ExitStack

import numpy as np

sys.path.insert(0, "/opt/trn_rl_repo")

import concourse.bass as bass  # noqa: E402
import concourse.tile as tile  # noqa: E402
from concourse import bacc, mybir  # noqa: E402
from concourse.bass_utils import run_bass_kernel_spmd  # noqa: E402

import ml_dtypes  # noqa: E402

BF16 = mybir.dt.bfloat16
F32 = mybir.dt.float32
AF = mybir.ActivationFunctionType
ALU = mybir.AluOpType

B, N, DIM = 8, 4096, 512
L, SEG = 256, 16
CT = DIM // 128
MT = N // 512

XT_CHUNK = 1024  # width of x DMA chunks (2KB rows = one DMA descriptor/run)
RSCALE = float(1.0 / np.sqrt(512.0))


def build_kernel(ctx: ExitStack, tc: "tile.TileContext", out_d, xt_d, wkv_d, wqT_d, wproj_d, bproj_d):
    nc = tc.nc

    consts = ctx.enter_context(tc.tile_pool(name="consts", bufs=1))
    work = ctx.enter_context(tc.tile_pool(name="work", bufs=3))
    psum = ctx.enter_context(tc.tile_pool(name="psum", bufs=4, space="PSUM"))
    psumv = ctx.enter_context(tc.tile_pool(name="psumv", bufs=2, space="PSUM"))
    psden = ctx.enter_context(tc.tile_pool(name="psden", bufs=2, space="PSUM"))

    # ---- weights ------------------------------------------------------------
    wkv = consts.tile([128, CT, 2 * DIM], BF16)  # [c_lo, cj, (k|v) columns]
    for j in range(CT):
        nc.sync.dma_start(out=wkv[:, j, :], in_=wkv_d[j, :, :])

    ones_col = consts.tile([128, 1], BF16)
    nc.vector.memset(ones_col[:, :], 1.0)
    ones_row = consts.tile([1, 128], BF16)
    nc.vector.memset(ones_row[:, :], 1.0)

    # ---- x^T (chunked DMA) + segment pooling on DVE -------------------------
    xt = consts.tile([128, CT, N], BF16)
    poolf = consts.tile([128, CT, L], F32)  # segment sums (= 16*mean), f32
    xpool = consts.tile([128, CT, L], BF16)
    NCH = N // XT_CHUNK
    LC = L // NCH
    for ci in range(NCH):
        c0, c1 = ci * XT_CHUNK, (ci + 1) * XT_CHUNK
        for j in range(CT):
            nc.sync.dma_start(out=xt[:, j, c0:c1], in_=xt_d[j, :, c0:c1])
        for j in range(CT):
            nc.vector.reduce_sum(
                poolf[:, j, ci * LC : (ci + 1) * LC],
                xt[:, j, c0:c1].rearrange("p (l s) -> p l s", s=SEG),
                axis=mybir.AxisListType.X,
            )
    for j in range(CT):
        nc.vector.tensor_scalar_mul(xpool[:, j, :], poolf[:, j, :], 1.0 / SEG)

    wqT = consts.tile([128, CT, DIM], BF16)  # W_q^T  [d_lo, dj, c]
    for j in range(CT):
        nc.sync.dma_start(out=wqT[:, j, :], in_=wqT_d[j, :, :])
    wproj = consts.tile([128, CT, DIM], BF16)
    for j in range(CT):
        nc.sync.dma_start(out=wproj[:, j, :], in_=wproj_d[j, :, :])
    bproj = consts.tile([1, DIM], BF16)
    nc.sync.dma_start(out=bproj[:, :], in_=bproj_d[:, :])

    # ---- v = x @ W_v, natural layout, first in program order so the PE is
    # busy from the first x chunk while the landmark chain waits on pooling.
    vstore = consts.tile([128, 32, 512], BF16)
    for mi in range(MT):
        for t in range(4):
            r0 = mi * 512 + t * 128
            vp = psumv.tile([128, 512], F32, tag="mmv")
            for cj in range(CT):
                nc.tensor.matmul(
                    vp[:, :],
                    xt[:, cj, r0 : r0 + 128],
                    wkv[:, cj, DIM : 2 * DIM],
                    start=(cj == 0),
                    stop=(cj == CT - 1),
                )
            # v copies on ACT (idle until the landmark chain at ~24us): on
            # DVE they queue behind all 16 pooling reduces, starving the
            # 2-slot v PSUM pool and stalling the PE 14.8us (trace-measured).
            # They precede the landmark copies in ACT order and finish
            # chasing the v matmuls, so the landmark chain is not delayed.
            nc.scalar.copy(vstore[:, mi * 4 + t, :], vp[:, :])

    # ---- landmark projections: keys_lm^T, vals_lm^T [d_lo, dj, l] ----------
    keysT = consts.tile([128, CT, L], BF16)
    valsT = consts.tile([128, CT, L], BF16)
    for dst, col0 in ((keysT, 0), (valsT, DIM)):
        for dj in range(CT):
            pt = psum.tile([128, L], F32, tag="mm")
            for cj in range(CT):
                nc.tensor.matmul(
                    pt[:, :],
                    wkv[:, cj, col0 + dj * 128 : col0 + (dj + 1) * 128],
                    xpool[:, cj, :],
                    start=(cj == 0),
                    stop=(cj == CT - 1),
                )
            nc.scalar.copy(dst[:, dj, :], pt[:, :])

    # ---- KW = W_q @ keys_lm^T  [c_lo, cj, l]  (the q-projection absorber) --
    kw = consts.tile([128, CT, L], BF16)
    for cj in range(CT):
        pt = psum.tile([128, L], F32, tag="mm")
        for dj in range(CT):
            nc.tensor.matmul(
                pt[:, :],
                wqT[:, dj, cj * 128 : (cj + 1) * 128],
                keysT[:, dj, :],
                start=(dj == 0),
                stop=(dj == CT - 1),
            )
        nc.scalar.copy(kw[:, cj, :], pt[:, :])

    # ---- VWb = vals_lm @ W_proj + 1b  [l_lo, li, d] -------------------------
    vw = consts.tile([128, 2, DIM], BF16)
    for li in range(2):
        pt = psum.tile([128, DIM], F32, tag="mm")
        for dj in range(CT):
            nc.tensor.matmul(
                pt[:, :],
                valsT[:, dj, li * 128 : (li + 1) * 128],
                wproj[:, dj, :],
                start=(dj == 0),
                stop=False,
            )
        nc.tensor.matmul(pt[:, :], ones_row[:, :], bproj[:, :], start=False, stop=True)
        nc.scalar.copy(vw[:, li, :], pt[:, :])

    # ---- attention: scoresT = KW^T-contracted directly against x^T ----------
    # software-pipelined: scores(mi+1) is issued before out2/den(mi) so the
    # PE never stalls on the exp (ACT) latency.
    def do_scores(mi):
        et = work.tile([128, 2, 512], BF16, tag="et")  # E = exp(logits/sqrt(512))
        for li in range(2):
            pt = psum.tile([128, 512], F32, tag="mm")
            for cj in range(CT):
                nc.tensor.matmul(
                    pt[:, :],
                    kw[:, cj, li * 128 : (li + 1) * 128],
                    xt[:, cj, mi * 512 : (mi + 1) * 512],
                    start=(cj == 0),
                    stop=(cj == CT - 1),
                )
            nc.scalar.activation(et[:, li, :], pt[:, :], AF.Exp, scale=RSCALE)
        return et

    et_next = do_scores(0)
    for mi in range(MT):
        et = et_next
        if mi + 1 < MT:
            et_next = do_scores(mi + 1)

        for t in range(4):
            r0 = mi * 512 + t * 128
            sl = slice(t * 128, (t + 1) * 128)
            # out2 = E @ VWb, then den with the same stationary weights.
            # out2 lives in the v-phase psum pool (idle during attention: the
            # in-order PE finished all v matmuls before the first score MM) -
            # this decouples out2's slot cycle (freed at DVE fin pace) from
            # the scores pipeline in the "mm" pool.
            op = psumv.tile([128, 512], F32, tag="mmv")
            dp = psden.tile([128, 1], F32, tag="den")
            for li in range(2):
                nc.tensor.matmul(
                    op[:, :], et[:, li, sl], vw[:, li, :],
                    start=(li == 0), stop=(li == 1),
                )
            for li in range(2):
                nc.tensor.matmul(
                    dp[:, :], et[:, li, sl], ones_col[:, :],
                    start=(li == 0), stop=(li == 1),
                )
            rr = work.tile([128, 1], F32, tag="rr")
            nc.vector.reciprocal(rr[:, :], dp[:, :])
            fin = work.tile([128, 512], F32, tag="fin")
            nc.vector.scalar_tensor_tensor(
                fin[:, :], op[:, :], rr[:, :], vstore[:, mi * 4 + t, :],
                op0=ALU.mult, op1=ALU.add,
            )
            nc.sync.dma_start(out=out_d[r0 : r0 + 128, :], in_=fin[:, :])


def build_nc(repeat: int = 1):
    nc = bacc.Bacc("TRN2", target_bir_lowering=False, debug=False, num_devices=8)
    xt_d = nc.declare_dram_parameter("xt", [CT, 128, N], BF16, isOutput=False)
    wkv_d = nc.declare_dram_parameter("wkv", [CT, 128, 2 * DIM], BF16, isOutput=False)
    wqT_d = nc.declare_dram_parameter("wqT", [CT, 128, DIM], BF16, isOutput=False)
    wproj_d = nc.declare_dram_parameter("wproj", [CT, 128, DIM], BF16, isOutput=False)
    bproj_d = nc.declare_dram_parameter("bproj", [1, DIM], BF16, isOutput=False)
    out_d = nc.declare_dram_parameter("out", [N, DIM], F32, isOutput=True)
    aps = (out_d.ap(), xt_d.ap(), wkv_d.ap(), wqT_d.ap(), wproj_d.ap(), bproj_d.ap())
    with tile.TileContext(nc) as tc, ExitStack() as ctx:
        if repeat == 1:
            build_kernel(ctx, tc, *aps)
        else:
            with tc.For_i(0, repeat, 1):
                build_kernel(ctx, tc, *aps)
    nc.compile()
    return nc


def prep_in_maps(x, W_qkv, W_proj, b_proj):
    bf = ml_dtypes.bfloat16
    W_qkv = np.asarray(W_qkv, np.float32)
    wkv = np.ascontiguousarray(W_qkv[:, DIM:].astype(bf).reshape(CT, 128, 2 * DIM))
    wqT = np.ascontiguousarray(W_qkv[:, :DIM].T.astype(bf)).reshape(CT, 128, DIM)
    wp = np.ascontiguousarray(np.asarray(W_proj, np.float32).astype(bf).reshape(CT, 128, DIM))
    bp = np.asarray(b_proj, np.float32).astype(bf).reshape(1, DIM)
    in_maps = []
    for i in range(B):
        xti = np.ascontiguousarray(np.asarray(x[i], np.float32).T.astype(bf)).reshape(CT, 128, N)
        in_maps.append({"xt": xti, "wkv": wkv, "wqT": wqT, "wproj": wp, "bproj": bp})
    return in_maps


_NC_CACHE = None


def kernel(x, W_qkv, W_proj, b_proj):
    global _NC_CACHE
    if _NC_CACHE is None:
        _NC_CACHE = build_nc()
    nc = _NC_CACHE
    in_maps = prep_in_maps(x, W_qkv, W_proj, b_proj)
    res = run_bass_kernel_spmd(nc, in_maps, core_ids=list(range(B)))
    out = np.stack([res.results[i]["out"] for i in range(B)], axis=0)
    return out.astype(np.float32)
